# revision 7
# baseline (speedup 1.0000x reference)
"""DualMambaBlock Trainium2 kernel: 8-core SPMD Bass/Tile implementation (v2).

Reference computes (B=4, L=256, C=32, D=128, DI=256, DS=16, DC=4, DR=8):
  T_out = temporal mamba over L (batch B*C)     -> [B,L,C,D]
  C_out = channel mamba over C (batch B*L) on gated x -> [B,L,C,D]
  gate g[b,c] = sigmoid(||mean_l(x) @ g_w + g_b||^2 / 8)

Sharding: core k handles b = k//2;  temporal: c in [16*(k%2), +16);
channel: l in [128*(k%2), +128).  Gate mean needs full L so each core also
reads the sibling half of x[b].

Device layout: everything is [d_partition(128) x tokens] ("transposed"),
tokens are (seq-major, t contiguous).  Host pre-transposes inputs and
post-transposes outputs, so all DMAs are contiguous.

v2 engine assignment (DVE is the bottleneck):
  - scans: DVE (tensor_tensor_scan, ~2.1cy/elem on HW)
  - in1 = dtu*B: Pool/GpSimd engine (tensor_mul)
  - hc = h*C: DVE (2x mode)
  - conv: DVE tsm + (tsm+add) per tap on packed 3D views (STT is 3x slower)
  - u*D skip: PE diag-matmul accumulated into y_ps
  - dA = exp(a_s*dt): Scalar
  - y accumulation over ds: PE identity matmul into PSUM
  - outputs + bc: DMA directly from PSUM
Phase order: t-A, c-A1 (proj+conv, no silu), t-B, c-A2 (silu+dt), c-B so
c-A1 overlaps t-B's scans.
"""
import numpy as np
import ml_dtypes

import concourse.bass as bass
import concourse.bacc as bacc
import concourse.tile as tile
import concourse.mybir as mybir
from concourse.bass_utils import run_bass_kernel_spmd

F32 = mybir.dt.float32
BF16 = mybir.dt.bfloat16
AF = mybir.ActivationFunctionType
ALU = mybir.AluOpType
BF = ml_dtypes.bfloat16

B, L, C, D = 4, 256, 32, 128
DI, DS, DC, DR = 256, 16, 4, 8
ST = 4096            # tokens per core per mamba
NCH = 2              # chunks over ST
F = ST // NCH        # 2048 tokens per chunk
POISON = 40.0        # dt value whose exp(-k*dt) underflows to 0 for k>=1

_CACHE = {}
TRACE = False


def _ap3(t, p_ap, d0, d1):
    """view [128, d0(step0), d1] of a [128, d1] tile (free broadcast)."""
    return bass.AP(p_ap.tensor, p_ap.offset, [p_ap.ap[0], [0, d0], p_ap.ap[1]])


def build_program():
    nc = bacc.Bacc('TRN2', target_bir_lowering=False, debug=False, num_devices=8)

    def din(name, shape, dt=BF16):
        return nc.dram_tensor(name, shape, dt, kind='ExternalInput').ap()

    def dout(name, shape, dt=F32):
        return nc.dram_tensor(name, shape, dt, kind='ExternalOutput').ap()

    # per-core data
    xt = din('xt', [128, ST])          # temporal tokens (c-major, l contig)
    xc = din('xc', [128, ST])          # channel tokens own l-half (l-major, c contig)
    xo = din('xo', [128, ST])          # other l-half (for gate mean only)
    t_out = dout('t_out', [128, ST])
    c_out = dout('c_out', [128, ST])

    # weights (shared across cores); layouts chosen for direct DMA as lhsT
    w = {}
    for p in ('t', 'c'):
        w[p + '_w_in'] = din(p + '_w_in', [D, 2 * DI])        # lhsT [K=128, 512]
        w[p + '_dteff'] = din(p + '_dteff', [DI, DI])         # lhsT [K=256, 256]
        w[p + '_w_bc'] = din(p + '_w_bc', [DI, 2 * DS])       # lhsT [K=256, 32]
        w[p + '_w_out'] = din(p + '_w_out', [DI, D])          # lhsT [K=256, 128]
        w[p + '_conv_w'] = din(p + '_conv_w', [D, 2 * DC], F32)  # [128, 8] col q*4+j
        w[p + '_conv_b'] = din(p + '_conv_b', [DI], F32)
        w[p + '_b_dt'] = din(p + '_b_dt', [DI], F32)
        w[p + '_ddiag'] = din(p + '_ddiag', [D, 2 * D])       # diag(d) blocks, bf16
        w[p + '_a'] = din(p + '_a', [DS], F32)                # -exp(a_log[0])
    w['g_w'] = din('g_w', [D, D // 2])                        # pre-scaled by 1/L
    w['g_b'] = din('g_b', [D // 2], F32)

    with tile.TileContext(nc) as tc:
        import contextlib
        with contextlib.ExitStack() as ctx:
            wp = ctx.enter_context(tc.tile_pool(name='wp', bufs=1))
            xp = ctx.enter_context(tc.tile_pool(name='xp', bufs=1))
            big = ctx.enter_context(tc.tile_pool(name='big', bufs=1))
            work = ctx.enter_context(tc.tile_pool(name='work', bufs=4))
            sm = ctx.enter_context(tc.tile_pool(name='sm', bufs=2))
            ps_a = ctx.enter_context(tc.tile_pool(name='ps_a', bufs=2, space='PSUM'))
            ps_y = ctx.enter_context(tc.tile_pool(name='ps_y', bufs=1, space='PSUM'))

            # ---- load weights ------------------------------------------------
            sb = {}
            for name, ap in w.items():
                shape = list(ap.shape)
                if len(shape) == 1 and shape[0] == DI:
                    # [256] vector -> [128, 2] tile; column q holds di-tile q
                    t = wp.tile([128, 2], ap.dtype, tag='w_' + name)
                    nc.sync.dma_start(
                        out=t[:],
                        in_=bass.AP(ap.tensor, ap.offset, [[1, 128], [128, 2]]))
                elif len(shape) == 1 and shape[0] == DS:
                    # [16] vector -> broadcast across 128 partitions
                    t = wp.tile([128, DS], ap.dtype, tag='w_' + name)
                    nc.sync.dma_start(
                        out=t[:],
                        in_=bass.AP(ap.tensor, ap.offset, [[0, 128], [1, DS]]))
                elif len(shape) == 1:
                    t = wp.tile([shape[0], 1], ap.dtype, tag='w_' + name)
                    nc.sync.dma_start(out=t[:], in_=ap[:, None])
                elif shape[0] > 128:
                    # split K=256 weights into two [128, N] tiles
                    t = []
                    for kq in range(2):
                        tt = wp.tile([128, shape[1]], ap.dtype,
                                     tag=f'w_{name}_{kq}', name=f'w_{name}_{kq}')
                        nc.sync.dma_start(out=tt[:],
                                          in_=ap[kq * 128:(kq + 1) * 128, :])
                        t.append(tt)
                else:
                    t = wp.tile(shape, ap.dtype, tag='w_' + name)
                    nc.sync.dma_start(out=t[:], in_=ap[:])
                sb[name] = t

            from concourse.masks import make_identity
            ones1 = wp.tile([1, 128], BF16, tag='ones1')
            nc.vector.memset(ones1[:], 1.0)
            ones64 = wp.tile([64, 1], BF16, tag='ones64')
            nc.vector.memset(ones64[:], 1.0)
            ident = wp.tile([128, 128], BF16, tag='ident')
            make_identity(nc, ident[:])

            # x tiles
            xt_s = xp.tile([128, ST], BF16, tag='xt')
            nc.sync.dma_start(out=xt_s[:], in_=xt[:])
            xc_s = xp.tile([128, ST], BF16, tag='xc')
            nc.sync.dma_start(out=xc_s[:], in_=xc[:])
            xo_s = xp.tile([128, ST], BF16, tag='xo')
            nc.sync.dma_start(out=xo_s[:], in_=xo[:])

            # ---- gate --------------------------------------------------------
            # mean over l: view [d; c, l] of l-major tokens (col = l*32+c)
            m1 = sm.tile([128, C], F32, tag='m1')
            m2 = sm.tile([128, C], F32, tag='m2')
            nc.vector.reduce_sum(
                m1[:], bass.AP(xc_s[:].tensor, xc_s[:].offset,
                               [xc_s[:].ap[0], [1, C], [C, 128]]),
                axis=mybir.AxisListType.X)
            nc.vector.reduce_sum(
                m2[:], bass.AP(xo_s[:].tensor, xo_s[:].offset,
                               [xo_s[:].ap[0], [1, C], [C, 128]]),
                axis=mybir.AxisListType.X)
            msum = sm.tile([128, C], BF16, tag='msum')
            nc.vector.tensor_add(msum[:], m1[:], m2[:])
            node_ps = ps_a.tile([64, C], F32, tag='mm')
            nc.tensor.matmul(node_ps[:], sb['g_w'][:], msum[:], start=True, stop=True)
            node_sq = sm.tile([64, C], BF16, tag='node_sq')
            nc.scalar.activation(node_sq[:], node_ps[:], AF.Square,
                                 bias=sb['g_b'][:], scale=1.0)
            nrm_ps = ps_a.tile([1, C], F32, tag='mm')
            nc.tensor.matmul(nrm_ps[:], ones64[:], node_sq[:], start=True, stop=True)
            g_row = sm.tile([1, C], BF16, tag='g_row')
            nc.scalar.activation(g_row[:], nrm_ps[:], AF.Sigmoid, scale=0.125)
            grep_ps = ps_a.tile([128, C], F32, tag='mm')
            nc.tensor.matmul(grep_ps[:], ones1[:], g_row[:], start=True, stop=True)
            g_tile = sm.tile([128, C], BF16, tag='g_tile')
            nc.scalar.copy(g_tile[:], grep_ps[:])
            # xg = xc * g (broadcast over l via step-0); reuse xo slot
            xg_s = xp.tile([128, ST], BF16, tag='xo')
            nc.vector.tensor_mul(
                xg_s[:].rearrange('p (l c) -> p l c', c=C),
                xc_s[:].rearrange('p (l c) -> p l c', c=C),
                _ap3(g_tile, g_tile[:], L // 2, C))

            # ---- mamba phase A: projections + conv ---------------------------
            def mamba_A(pfx, xsrc, T, tags, do_silu_dt):
                w_in = sb[pfx + '_w_in']
                conv_w = sb[pfx + '_conv_w']
                conv_b = sb[pfx + '_conv_b']

                u = [big.tile([128, ST], BF16, tag=tags['u'][q], name=f'{pfx}u{q}')
                     for q in range(2)]
                uc = [big.tile([128, ST], BF16, tag=tags['uc'][q], name=f'{pfx}uc{q}')
                      for q in range(2)]
                zs = [big.tile([128, ST], BF16, tag=tags['zs'][q], name=f'{pfx}zs{q}')
                      for q in range(2)]

                # projections, full width in 512-col steps
                for q in range(2):
                    for j in range(ST // 512):
                        cols = slice(j * 512, (j + 1) * 512)
                        up = ps_a.tile([128, 512], F32, tag='mm')
                        nc.tensor.matmul(up[:], w_in[:, q * 128:(q + 1) * 128],
                                         xsrc[:, cols], start=True, stop=True)
                        nc.scalar.copy(uc[q][:, cols], up[:])
                        zp = ps_a.tile([128, 512], F32, tag='mm')
                        nc.tensor.matmul(zp[:], w_in[:, 256 + q * 128:256 + (q + 1) * 128],
                                         xsrc[:, cols], start=True, stop=True)
                        if do_silu_dt:
                            nc.scalar.activation(zs[q][:, cols], zp[:], AF.Silu)
                        else:
                            nc.scalar.copy(zs[q][:, cols], zp[:])
                # causal depthwise conv along t within each seq:
                # out[t] = sum_j w[j] * u[t-3+j],  kernel index j=3 is "no shift"
                for q in range(2):
                    for c9 in range(NCH):
                        c9s = slice(c9 * F, (c9 + 1) * F)
                        nc.vector.tensor_scalar_mul(u[q][:, c9s], uc[q][:, c9s],
                                                    conv_w[:, q * 4 + 3:q * 4 + 4])
                        for j in range(3):
                            sh = 3 - j  # shift amount
                            o_v = u[q][:, c9s].rearrange(
                                'p (s t) -> p s t', t=T)[:, :, sh:T]
                            i_v = uc[q][:, c9s].rearrange(
                                'p (s t) -> p s t', t=T)[:, :, 0:T - sh]
                            ctmp = work.tile([128, F], BF16, tag='ctmp',
                                             name='ctmp', bufs=1)
                            t_v = ctmp[:].rearrange(
                                'p (s t) -> p s t', t=T)[:, :, 0:T - sh]
                            nc.vector.tensor_scalar_mul(
                                t_v, i_v, conv_w[:, q * 4 + j:q * 4 + j + 1])
                            nc.vector.tensor_add(o_v, o_v, t_v)
                        if do_silu_dt:
                            nc.scalar.activation(u[q][:, c9s], u[q][:, c9s], AF.Silu,
                                                 bias=conv_b[:, q:q + 1], scale=1.0)
                return u, uc, zs

            # ---- mamba phase A2: silu (if deferred) + dt + dtu + bc ----------
            def mamba_A2(pfx, T, tags, u, zs, did_silu):
                dteff = sb[pfx + '_dteff']
                w_bc = sb[pfx + '_w_bc']
                b_dt = sb[pfx + '_b_dt']
                conv_b = sb[pfx + '_conv_b']

                if not did_silu:
                    for q in range(2):
                        nc.scalar.activation(u[q][:], u[q][:], AF.Silu,
                                             bias=conv_b[:, q:q + 1], scale=1.0)
                        nc.scalar.activation(zs[q][:], zs[q][:], AF.Silu)

                dt = [big.tile([128, ST], BF16, tag=tags['dt'][q], name=f'{pfx}dt{q}')
                      for q in range(2)]
                dtu = [big.tile([128, ST], BF16, tag=tags['dtu'][q], name=f'{pfx}du{q}')
                       for q in range(2)]
                # dt projection (K=256) + softplus = ln(1 + exp(.)); Exp per
                # 512 psum chunk into a bf16 stage, then one wide Ln -- keeps
                # exp/ln table switches rare.  bf16 et is safe: d(dt) =
                # d(et)/(1+et) <= 0.004*et/(1+et), and the scan's
                # amplification of dt error cancels the et/(1+et) factor.
                for q in range(2):
                    for g in range(NCH):
                        gcols = slice(g * F, (g + 1) * F)
                        et = work.tile([128, F], BF16, tag='et', name='et', bufs=1)
                        for j in range(F // 512):
                            cols = slice(g * F + j * 512, g * F + (j + 1) * 512)
                            dp = ps_a.tile([128, 512], F32, tag='mm')
                            nc.tensor.matmul(dp[:], dteff[0][:, q * 128:(q + 1) * 128],
                                             u[0][:, cols], start=True, stop=False)
                            nc.tensor.matmul(dp[:], dteff[1][:, q * 128:(q + 1) * 128],
                                             u[1][:, cols], start=False, stop=True)
                            nc.scalar.activation(et[:, j * 512:(j + 1) * 512], dp[:],
                                                 AF.Exp, bias=b_dt[:, q:q + 1],
                                                 scale=1.0)
                        nc.scalar.activation(dt[q][:, gcols], et[:], AF.Ln, bias=1.0)
                    nc.vector.tensor_mul(dtu[q][:], dt[q][:], u[q][:])
                    # poison seq starts so exp(-k*dt) == 0 there (scan reset)
                    nc.vector.memset(
                        dt[q][:].rearrange('p (s t) -> p s t', t=T)[:, :, 0:1],
                        POISON)
                # B/C compact projection [32, ST], straight to DRAM from PSUM
                bc_dram = nc.dram_tensor(f'{pfx}_bc_scratch', [2 * DS, ST],
                                         BF16).ap()
                for j in range(ST // 512):
                    cols = slice(j * 512, (j + 1) * 512)
                    bp = ps_a.tile([32, 512], F32, tag='bc')
                    nc.tensor.matmul(bp[:], w_bc[0][:], u[0][:, cols],
                                     start=True, stop=False)
                    nc.tensor.matmul(bp[:], w_bc[1][:], u[1][:, cols],
                                     start=False, stop=True)
                    bb = sm.tile([32, 512], BF16, tag='bb', name='bb', bufs=2)
                    nc.scalar.copy(bb[:], bp[:])
                    nc.sync.dma_start(out=bc_dram[:, cols], in_=bb[:])
                return dt, dtu, bc_dram

            # ---- mamba phase B: chunked ds scan loop -------------------------
            def mamba_B(pfx, T, u, zs, dt, dtu, bc_dram, out_dram):
                ddiag = sb[pfx + '_ddiag']
                a_vec = sb[pfx + '_a']
                for ch in range(NCH):
                    cols = slice(ch * F, (ch + 1) * F)
                    y2 = []
                    for q in range(2):
                        y_ps = ps_y.tile([128, F], F32, tag='y_ps', name='y_ps')
                        # skip-path u*D via diag matmul (initializes PSUM)
                        for j in range(F // 512):
                            o2 = slice(j * 512, (j + 1) * 512)
                            c2 = slice(ch * F + j * 512, ch * F + (j + 1) * 512)
                            nc.tensor.matmul(y_ps[:, o2],
                                             ddiag[:, q * 128:(q + 1) * 128],
                                             u[q][:, c2], start=True, stop=False)
                        for ds in range(DS):
                            brep = work.tile([128, F], BF16, tag='brep',
                                             name='brep', bufs=2)
                            crep = work.tile([128, F], BF16, tag='crep',
                                             name='crep', bufs=2)
                            nc.sync.dma_start(
                                out=brep[:],
                                in_=bass.AP(bc_dram.tensor, ds * ST + ch * F,
                                            [[0, 128], [1, F]]))
                            nc.sync.dma_start(
                                out=crep[:],
                                in_=bass.AP(bc_dram.tensor, (DS + ds) * ST + ch * F,
                                            [[0, 128], [1, F]]))
                            dA = work.tile([128, F], BF16, tag='dA', name='dA',
                                           bufs=2)
                            nc.scalar.activation(dA[:], dt[q][:, cols], AF.Exp,
                                                 scale=a_vec[:, ds:ds + 1])
                            in1 = work.tile([128, F], BF16, tag='in1', name='in1',
                                            bufs=3)
                            nc.gpsimd.tensor_mul(in1[:], dtu[q][:, cols], brep[:])
                            h = work.tile([128, F], BF16, tag='h', name='h', bufs=2)
                            nc.vector.tensor_tensor_scan(
                                h[:], dA[:], in1[:], 0.0,
                                op0=ALU.mult, op1=ALU.add)
                            hc = work.tile([128, F], BF16, tag='hc', name='hc',
                                           bufs=1)
                            nc.vector.tensor_mul(hc[:], h[:], crep[:])
                            for j in range(F // 512):
                                o2 = slice(j * 512, (j + 1) * 512)
                                nc.tensor.matmul(y_ps[:, o2], ident[:], hc[:, o2],
                                                 start=False, stop=(ds == DS - 1))
                        # finalize: y2 = (y_scan + u*D) * silu(z); lands in dtu
                        # cols (dtu is fully consumed for this chunk by now)
                        y2q = dtu[q][:, cols]
                        nc.vector.tensor_mul(y2q, y_ps[:], zs[q][:, cols])
                        y2.append(y2q)
                    w_out = sb[pfx + '_w_out']
                    for j in range(F // 512):
                        c2 = slice(ch * F + j * 512, ch * F + (j + 1) * 512)
                        jj = slice(j * 512, (j + 1) * 512)
                        op = ps_a.tile([128, 512], F32, tag='mm')
                        nc.tensor.matmul(op[:], w_out[0][:], y2[0][:, jj],
                                         start=True, stop=False)
                        nc.tensor.matmul(op[:], w_out[1][:], y2[1][:, jj],
                                         start=False, stop=True)
                        ot = work.tile([128, 512], F32, tag='ot', name='ot', bufs=2)
                        nc.scalar.copy(ot[:], op[:])
                        nc.sync.dma_start(out=out_dram[:, c2], in_=ot[:])

            t_tags = {'u': ['u0', 'u1'], 'uc': ['cu0', 'cu1'],
                      'zs': ['zs0', 'zs1'], 'dt': ['dt0', 'dt1'],
                      'dtu': ['du0', 'du1']}
            c_tags = {'u': ['cu0', 'cu1'], 'uc': ['xt', 'xc'],
                      'zs': ['cz0', 'cz1'], 'dt': ['dt0', 'dt1'],
                      'dtu': ['du0', 'du1']}

            # t-A (with silu+dt inline), c-A1 (defer silu), t-B, c-A2, c-B
            t_u, t_uc, t_zs = mamba_A('t', xt_s, L, t_tags, do_silu_dt=True)
            t_dt, t_dtu, t_bc = mamba_A2('t', L, t_tags, t_u, t_zs, did_silu=True)
            c_u, c_uc, c_zs = mamba_A('c', xg_s, C, c_tags, do_silu_dt=False)
            mamba_B('t', L, t_u, t_zs, t_dt, t_dtu, t_bc, t_out)
            c_dt, c_dtu, c_bc = mamba_A2('c', C, c_tags, c_u, c_zs, did_silu=False)
            mamba_B('c', C, c_u, c_zs, c_dt, c_dtu, c_bc, c_out)

    nc.compile()
    return nc


def _shard_host(inputs):
    """Build per-core input maps from full inputs."""
    x = np.asarray(inputs['x'], np.float32)

    def prep(pfx):
        w_in = np.asarray(inputs[pfx + 'w_in'], np.float32)
        w_xproj = np.asarray(inputs[pfx + 'w_xproj'], np.float32)
        w_dt = np.asarray(inputs[pfx + 'w_dt'], np.float32)
        dteff = w_xproj[:, :DR] @ w_dt
        conv_w = np.asarray(inputs[pfx + 'conv_w'], np.float32).reshape(DC, DI)
        conv_w = np.ascontiguousarray(
            conv_w.reshape(DC, 2, D).transpose(2, 1, 0).reshape(D, 2 * DC))
        a_vec = -np.exp(np.asarray(inputs[pfx + 'a_log'], np.float32)[0])
        d_skip = np.asarray(inputs[pfx + 'd'], np.float32)
        ddiag = np.zeros((D, 2 * D), np.float32)
        for q in range(2):
            np.fill_diagonal(ddiag[:, q * D:(q + 1) * D], d_skip[q * D:(q + 1) * D])
        return {
            pfx + 'w_in': w_in.astype(BF),
            pfx + 'dteff': dteff.astype(BF),
            pfx + 'w_bc': w_xproj[:, DR:].astype(BF),
            pfx + 'w_out': np.asarray(inputs[pfx + 'w_out'], np.float32).astype(BF),
            pfx + 'conv_w': conv_w,
            pfx + 'conv_b': np.asarray(inputs[pfx + 'conv_b'], np.float32),
            pfx + 'b_dt': np.asarray(inputs[pfx + 'b_dt'], np.float32),
            pfx + 'ddiag': ddiag.astype(BF),
            pfx + 'a': a_vec,
        }

    shared = {}
    shared.update(prep('t_'))
    shared.update(prep('c_'))
    shared['g_w'] = (np.asarray(inputs['g_w_node'], np.float32) / L).astype(BF)
    shared['g_b'] = np.asarray(inputs['g_b_node'], np.float32)

    in_maps = []
    for k in range(8):
        b, half = k // 2, k % 2
        # temporal tokens: c-major within c-half -> [d, c*L + l]
        xt = x[b, :, 16 * half:16 * (half + 1), :]          # [L, 16, D]
        xt = np.ascontiguousarray(xt.transpose(2, 1, 0).reshape(D, ST))
        # channel tokens own half: l-major -> [d, l*C + c]
        xch = x[b, 128 * half:128 * (half + 1)]             # [128, C, D]
        xch = np.ascontiguousarray(xch.transpose(2, 0, 1).reshape(D, ST))
        xoh = x[b, 128 * (1 - half):128 * (2 - half)]
        xoh = np.ascontiguousarray(xoh.transpose(2, 0, 1).reshape(D, ST))
        m = dict(shared)
        m['xt'] = xt.astype(BF)
        m['xc'] = xch.astype(BF)
        m['xo'] = xoh.astype(BF)
        in_maps.append(m)
    return in_maps


def kernel(**inputs):
    if 'nc' not in _CACHE:
        _CACHE['nc'] = build_program()
    nc = _CACHE['nc']
    in_maps = _shard_host(inputs)
    res = run_bass_kernel_spmd(nc, in_maps, list(range(8)), trace=TRACE)
    _CACHE['last_result'] = res

    T_out = np.zeros((B, L, C, D), np.float32)
    C_out = np.zeros((B, L, C, D), np.float32)
    for k in range(8):
        b, half = k // 2, k % 2
        to = res.results[k]['t_out']          # [d, c*L + l]
        T_out[b, :, 16 * half:16 * (half + 1), :] = \
            to.reshape(D, 16, L).transpose(2, 1, 0)
        co = res.results[k]['c_out']          # [d, l*C + c]
        C_out[b, 128 * half:128 * (half + 1)] = \
            co.reshape(D, 128, C).transpose(1, 2, 0)
    return (T_out, C_out)
